# revision 46
# baseline (speedup 1.0000x reference)
"""Trainium2 Bass kernel for a 4-layer Mamba (BioSeqMixer) model.

Sharding: 8 cores = (batch 4) x (d_inner half 2). Each core runs the full
4-layer stack for one batch element over its 768-channel half of d_inner.
Cross-core traffic per layer: pair all-reduce of the x_proj partial
([80,512] f16, per t-half) and of the out_proj partial ([768,512] f16 per
t-half), pipelined against compute of the other t-half.

Scan block: channels on partitions, time on the free axis; 16 independent
tensor_tensor_scan recurrences (one per state n), merged full-width per
(state, t-half) with dA=0 chunk-boundary resets (half 0) or injected
half-0 final states (half 1). y = sum_n h*C is accumulated on the PE via
batched identity matmuls into PSUM (D*xs folded in via a diag(D) matmul).
conv runs on DVE (fills the x_proj-AR wait), h*C products go to the Pool
engine, LN stats run on the ACT accumulator per t-half, and each layer's
second-half residual add is deferred into the next layer's emission so
the out-AR flight never blocks DVE's in-order queue.
"""

import sys

sys.path.insert(0, "/opt/trn_rl_repo")

import numpy as np

import concourse.bass as bass
import concourse.bacc as bacc
import concourse.mybir as mybir
import concourse.tile as tile
from concourse.bass_utils import run_bass_kernel_spmd

# model dims
B, L = 4, 1024
DM, NL, VOCAB = 768, 4, 8
DI, NST, DCONV, RDT = 1536, 16, 4, 48
EPS = 1e-5

# per-core dims
T = L            # tokens per core (one batch element)
H = T // 2       # t-half length
DH = DI // 2     # d_inner half per core
NDC = DH // 128  # d-chunks (6)
NMC = DM // 128  # d_model chunks (6)
NTC = T // 128   # token chunks (8)
G = 3            # d-chunks per group
NG = NDC // G    # groups (2)
GW = G * H       # group width (1536)

N_CORES = 8

F32 = mybir.dt.float32
F16 = mybir.dt.float16
AF = mybir.ActivationFunctionType
ALU = mybir.AluOpType

# knobs
UB_POOL_N = 0    # uB on Pool gates the serial scan chain - keep on DVE
HC_POOL_N = 6    # h*C mults on Pool only feed the PE accs, safe to offload
CONV_DVE_HF0 = True  # conv for t-half 0 on DVE (fills pre-scan DVE idle)
BCAST_MODE = "pool"   # 'pool' | 'dma'
CC_DT = F16      # dtype for the out_proj / x_proj allreduce
SKIP_CC = False  # replace collectives with local copies (for TimelineSim)


def _np16(x):
    return np.ascontiguousarray(x, dtype=np.float16)


def _np32(x):
    return np.ascontiguousarray(x, dtype=np.float32)


def detect_a_scalars(A_log):
    """If A[d, n] is d-independent for every (layer, n), return the
    per-(layer, n) scalar values; else None."""
    A = -np.exp(np.asarray(A_log, np.float64))  # (NL, DI, NST)
    scal = np.zeros((NL, NST), np.float64)
    for l in range(NL):
        for j in range(NST):
            col = A[l][:, j]
            if np.max(np.abs(col - col[0])) > 1e-6 * max(1.0, abs(col[0])):
                return None
            scal[l, j] = col[0]
    return scal


def prepare_host_inputs(inputs):
    """Returns per-core input dicts (host-side weight prep + sharding)."""
    embed = np.asarray(inputs["embed"], np.float32)
    input_ids = np.asarray(inputs["input_ids"])
    in_proj_w = np.asarray(inputs["in_proj_w"], np.float32)
    conv_w = np.asarray(inputs["conv_w"], np.float32)
    conv_b = np.asarray(inputs["conv_b"], np.float32)
    x_proj_w = np.asarray(inputs["x_proj_w"], np.float32)
    dt_proj_w = np.asarray(inputs["dt_proj_w"], np.float32)
    dt_proj_b = np.asarray(inputs["dt_proj_b"], np.float32)
    A_log = np.asarray(inputs["A_log"], np.float32)
    Dp = np.asarray(inputs["D"], np.float32)
    out_proj_w = np.asarray(inputs["out_proj_w"], np.float32)
    norm_w = np.asarray(inputs["norm_w"], np.float32)
    norm_b = np.asarray(inputs["norm_b"], np.float32)
    norm_f_w = np.asarray(inputs["norm_f_w"], np.float32)
    norm_f_b = np.asarray(inputs["norm_f_b"], np.float32)

    hidden0 = embed[input_ids]  # (B, L, DM)

    per_half = [{}, {}]
    for h in (0, 1):
        S = slice(h * DH, (h + 1) * DH)
        winx_t = np.empty((NL, DM, DH), np.float16)
        winz_t = np.empty((NL, DM, DH), np.float16)
        bxz = np.empty((NL, 2, DH), np.float32)
        vecs_extra = np.empty((NL, 6, DH), np.float32)  # convb,bdt,convw0..3
        wxp_t = np.empty((NL, DH, 80), np.float16)
        wdt_t = np.empty((NL, RDT, DH), np.float16)
        amat = np.empty((NL, DH, NST), np.float32)
        ddiag = np.empty((NL, NDC * 128, 128), np.float16)
        wout_t = np.empty((NL, DH, DM), np.float16)
        for l in range(NL):
            wx_rows = in_proj_w[l][:DI][S]          # (DH, DM)
            wz_rows = in_proj_w[l][DI:][S]          # (DH, DM)
            winx_t[l] = _np16((wx_rows * norm_w[l][None, :]).T)
            winz_t[l] = _np16((wz_rows * norm_w[l][None, :]).T)
            bxz[l, 0] = wx_rows @ norm_b[l]
            bxz[l, 1] = wz_rows @ norm_b[l]
            vecs_extra[l, 0] = conv_b[l][S]
            vecs_extra[l, 1] = dt_proj_b[l][S]
            for k in range(DCONV):
                vecs_extra[l, 2 + k] = conv_w[l][S][:, k]
            wxp_t[l] = _np16(x_proj_w[l][:, S].T)   # (DH, 80)
            wdt_t[l] = _np16(dt_proj_w[l][S].T)     # (RDT, DH)
            amat[l] = -np.exp(A_log[l][S])          # (DH, NST)
            for dc in range(NDC):
                dvals = Dp[l][S][dc * 128:(dc + 1) * 128]
                ddiag[l, dc * 128:(dc + 1) * 128] = _np16(np.diag(dvals))
            wout_t[l] = _np16(out_proj_w[l][:, S].T)  # (DH, DM)
        per_half[h] = dict(
            winx_t=winx_t, winz_t=winz_t, bxz=bxz, vecs_extra=vecs_extra,
            wxp_t=wxp_t, wdt_t=wdt_t, amat=amat, ddiag=ddiag, wout_t=wout_t,
        )

    wfin = _np16(np.tile(norm_f_w[None, :], (128, 1)))
    bfin = _np16(np.tile(norm_f_b[None, :], (128, 1)))
    ident = _np16(np.eye(128))

    in_maps = []
    for r in range(N_CORES):
        b, h = r // 2, r % 2
        m = dict(per_half[h])
        m = {k: np.ascontiguousarray(v) for k, v in m.items()}
        m["hidden0"] = _np16(hidden0[b])
        m["wfin"] = wfin
        m["bfin"] = bfin
        m["ident"] = ident
        in_maps.append(m)
    return in_maps


def build_program(a_scalars=None):
    nc = bacc.Bacc("TRN2", target_bir_lowering=False, debug=False,
                   num_devices=N_CORES)

    dt_in = {}

    def din(name, shape, dt=F32):
        dt_in[name] = nc.dram_tensor(name, list(shape), dt,
                                     kind="ExternalInput").ap()
        return dt_in[name]

    din("hidden0", (T, DM), F16)
    din("winx_t", (NL, DM, DH), F16)
    din("winz_t", (NL, DM, DH), F16)
    din("bxz", (NL, 2, DH))
    din("vecs_extra", (NL, 6, DH))
    din("wxp_t", (NL, DH, 80), F16)
    din("wdt_t", (NL, RDT, DH), F16)
    din("amat", (NL, DH, NST))
    din("ddiag", (NL, NDC * 128, 128), F16)
    din("wout_t", (NL, DH, DM), F16)
    din("wfin", (128, DM), F16)
    din("bfin", (128, DM), F16)
    din("ident", (128, 128), F16)

    out_ap = nc.dram_tensor("out", [T, DM], F32, kind="ExternalOutput").ap()

    with tile.TileContext(nc) as tc:
        _body(nc, tc, dt_in, out_ap, a_scalars)

    nc.compile()
    return nc


def _body(nc, tc, din, out_ap, a_scalars):
    import contextlib
    with contextlib.ExitStack() as ctx:
        _body_inner(ctx, nc, tc, din, out_ap, a_scalars)


def _rep_mid(ap, n):
    """[P, w] AP -> [P, n, w] with a step-0 broadcast middle dim."""
    return bass.AP(ap.tensor, ap.offset, [ap.ap[0], [0, n], ap.ap[1]])


def _s3(t, base_off, stride, nch, w):
    """3D AP [128, nch, w] into a persist tile at base_off with chunk
    stride `stride`."""
    b = t[:, base_off:base_off + 1]
    return bass.AP(b.tensor, b.offset, [b.ap[0], [stride, nch], [1, w]])


def _cols(t, start, stride, count):
    """Strided single-column AP [128, count] (1 elem per step)."""
    b = t[:, start:start + 1]
    return bass.AP(b.tensor, b.offset, [b.ap[0], [stride, count]])


def _ln_stats(nc, res, stats, sq, epsc):
    """Per token-chunk mean (mu) and 1/std (rstd) via ACT accumulators."""
    for c in range(NTC):
        rc = res[:, c * DM:(c + 1) * DM]
        nc.scalar.activation(sq[:, :DM], rc, AF.Identity,
                             accum_out=stats[:, c:c + 1])
        nc.scalar.activation(sq[:, :DM], rc, AF.Square,
                             accum_out=stats[:, NTC + c:NTC + c + 1])
    mu = stats[:, 2 * NTC:3 * NTC]
    rstd = stats[:, 3 * NTC:4 * NTC]
    nc.vector.tensor_scalar(mu, stats[:, 0:NTC], 1.0 / DM, None, ALU.mult)
    nc.vector.tensor_tensor(rstd, mu, mu, ALU.mult)
    nc.vector.scalar_tensor_tensor(rstd, stats[:, NTC:2 * NTC], 1.0 / DM,
                                   rstd, ALU.mult, ALU.subtract)
    nc.scalar.activation(rstd, rstd, AF.Ln, bias=epsc[:])
    nc.scalar.activation(rstd, rstd, AF.Exp, scale=-0.5)
    return mu, rstd


def _body_inner(ctx, nc, tc, din, out_ap, a_scalars):
    E = ctx.enter_context

    # pools
    persist = E(tc.tile_pool(name="persist", bufs=1))
    wpool = E(tc.tile_pool(name="weights", bufs=2))
    wsmall = E(tc.tile_pool(name="wsmall", bufs=1))
    lnt_pool = E(tc.tile_pool(name="lnt", bufs=1))
    scratch = E(tc.tile_pool(name="scratch", bufs=2))
    scr1 = E(tc.tile_pool(name="scr1", bufs=1))
    bcpool = E(tc.tile_pool(name="bc", bufs=1))
    sring = E(tc.tile_pool(name="sring", bufs=4))   # da/ub/hall ring
    upool = E(tc.tile_pool(name="upool", bufs=1))
    hcpool = E(tc.tile_pool(name="hc", bufs=3))
    yhpool = E(tc.tile_pool(name="yh", bufs=1))
    stpool = E(tc.tile_pool(name="stg", bufs=1))
    smalls = E(tc.tile_pool(name="smalls", bufs=1))
    tiny = E(tc.tile_pool(name="tiny", bufs=4))
    ps_mm = E(tc.tile_pool(name="ps_mm", bufs=1, space="PSUM"))
    ps_tr = E(tc.tile_pool(name="ps_tr", bufs=1, space="PSUM"))
    ps_y = E(tc.tile_pool(name="ps_y", bufs=1, space="PSUM"))
    dram = E(tc.tile_pool(name="dram", bufs=2, space="DRAM"))

    # persistent tiles
    res = persist.tile([128, NTC * DM], F16, tag="res")     # residual [t,dm]
    xbuf = persist.tile([128, NDC * (T + 3)], F16, tag="xbuf")  # conv in/out
    zs = persist.tile([128, NDC * T], F16, tag="zs")        # silu(z)
    dts = persist.tile([128, NDC * T], F16, tag="dts")      # softplus dt
    dbc16 = persist.tile([80, T], F16, tag="dbc16")
    hfin = persist.tile([128, NST * NDC], F32, tag="hfin")  # h finals half0
    ident_sb = persist.tile([128, 128], F16, tag="ident")
    wfin_sb = persist.tile([128, DM], F16, tag="wfin")
    bfin_sb = persist.tile([128, DM], F16, tag="bfin")
    epsc = persist.tile([128, 1], F32, tag="epsc")

    nc.vector.memset(epsc[:], EPS)
    nc.sync.dma_start(ident_sb[:], din["ident"][:, :])
    nc.sync.dma_start(wfin_sb[:], din["wfin"][:, :])
    nc.sync.dma_start(bfin_sb[:], din["bfin"][:, :])

    # residual <- hidden0 ([T, DM] -> [128, (tc dm)])
    nc.sync.dma_start(
        res[:].rearrange("p (c m) -> p c m", c=NTC),
        din["hidden0"].rearrange("(c p) m -> p c m", p=128))

    # zero the 3-column conv pads once
    for dc in range(NDC):
        nc.vector.memset(xbuf[:, dc * (T + 3): dc * (T + 3) + 3], 0.0)

    kw = dict(res=res, xbuf=xbuf, zs=zs, dts=dts, dbc16=dbc16,
              hfin=hfin, ident_sb=ident_sb, epsc=epsc,
              wpool=wpool, wsmall=wsmall, lnt_pool=lnt_pool, scratch=scratch,
              scr1=scr1,
              bcpool=bcpool, sring=sring, upool=upool,
              hcpool=hcpool, yhpool=yhpool, stpool=stpool, smalls=smalls,
              tiny=tiny, ps_mm=ps_mm, ps_tr=ps_tr, ps_y=ps_y, dram=dram)
    pending = None
    for layer in range(NL):
        asc = None if a_scalars is None else a_scalars[layer]
        pending = _layer(nc, tc, din, layer, asc, pending, **kw)
    pending()

    # final layernorm -> out
    stats = smalls.tile([128, 4 * NTC], F32, tag="stats")
    sq = scratch.tile([128, DM], F16, tag="sq")
    mu, rstd = _ln_stats(nc, res, stats, sq, epsc)
    for c in range(NTC):
        rc = res[:, c * DM:(c + 1) * DM]
        ot = scr1.tile([128, DM], F32, tag="lnout")
        nc.vector.tensor_scalar(ot[:], rc, mu[:, c:c + 1], rstd[:, c:c + 1],
                                ALU.subtract, ALU.mult)
        nc.vector.tensor_tensor(ot[:], ot[:], wfin_sb[:], ALU.mult)
        nc.vector.tensor_tensor(ot[:], ot[:], bfin_sb[:], ALU.add)
        nc.sync.dma_start(out_ap[c * 128:(c + 1) * 128, :], ot[:])


def _layer(nc, tc, din, layer, asc, pending, *, res, xbuf, zs, dts, dbc16, hfin,
           ident_sb, epsc, wpool, wsmall, lnt_pool, scratch, scr1, bcpool,
           sring, upool, hcpool, yhpool, stpool, smalls, tiny,
           ps_mm, ps_tr, ps_y, dram):
    lt = lambda name: din[name][layer]
    TP3 = T + 3

    def load3(tile_ap, dram_ap, k):
        nc.sync.dma_start(
            tile_ap.rearrange("p (k m) -> p k m", k=k),
            dram_ap.rearrange("(k p) m -> p k m", p=128))

    # --- load weights to sbuf ---
    winx = wpool.tile([128, NMC * DH], F16, tag="wbig")
    load3(winx[:], lt("winx_t"), NMC)
    winz = wpool.tile([128, NMC * DH], F16, tag="wbig")
    load3(winz[:], lt("winz_t"), NMC)
    wxp = wsmall.tile([128, NDC * 80], F16, tag="wxp")
    load3(wxp[:], lt("wxp_t"), NDC)
    wdt = wsmall.tile([RDT, DH], F16, tag="wdt")
    nc.sync.dma_start(wdt[:], lt("wdt_t")[:, :])
    ddiag = wsmall.tile([128, NDC * 128], F16, tag="ddiag")
    load3(ddiag[:], lt("ddiag"), NDC)
    amat = None
    if asc is None:
        amat = wsmall.tile([128, NDC * NST], F32, tag="amat")
        load3(amat[:], lt("amat"), NDC)
    vecs = wsmall.tile([128, NDC * 8], F32, tag="vecs")
    # layout per dchunk: [bx, bz, convb, bdt, convw0..3]
    nc.sync.dma_start(
        vecs[:, 0:NDC * 2].rearrange("p (b k) -> p b k", b=2),
        lt("bxz").rearrange("b (k p) -> p b k", p=128))
    nc.sync.dma_start(
        vecs[:, NDC * 2:NDC * 8].rearrange("p (v k) -> p v k", v=6),
        lt("vecs_extra").rearrange("v (k p) -> p v k", p=128))
    bx_c = lambda dc: vecs[:, dc:dc + 1]
    bz_c = lambda dc: vecs[:, NDC + dc:NDC + dc + 1]
    convb_c = lambda dc: vecs[:, NDC * 2 + dc:NDC * 2 + dc + 1]
    bdt_c = lambda dc: vecs[:, NDC * 3 + dc:NDC * 3 + dc + 1]
    convw_c = lambda k, dc: vecs[:, NDC * (4 + k) + dc:NDC * (4 + k) + dc + 1]

    # --- layernorm stats + apply, per t-half (so next-layer work for the
    # first half can start while the second half's out-AR is in flight) ---
    stats = smalls.tile([128, 4 * NTC], F32, tag="stats")
    sq = scr1.tile([128, DM], F16, tag="sq")
    lnT = lnt_pool.tile([128, NMC * T], F16, tag="lnT")
    lnT3 = lnT[:].rearrange("p (m t) -> p m t", m=NMC)

    def ln_half(hf):
        c0 = hf * 4
        for c in range(c0, c0 + 4):
            rc = res[:, c * DM:(c + 1) * DM]
            nc.scalar.activation(sq[:, :DM], rc, AF.Identity,
                                 accum_out=stats[:, c:c + 1])
            nc.scalar.activation(sq[:, :DM], rc, AF.Square,
                                 accum_out=stats[:, NTC + c:NTC + c + 1])
        mu = stats[:, 2 * NTC + c0:2 * NTC + c0 + 4]
        rstd = stats[:, 3 * NTC + c0:3 * NTC + c0 + 4]
        nc.vector.tensor_scalar(mu, stats[:, c0:c0 + 4], 1.0 / DM, None,
                                ALU.mult)
        nc.vector.tensor_tensor(rstd, mu, mu, ALU.mult)
        nc.vector.scalar_tensor_tensor(rstd, stats[:, NTC + c0:NTC + c0 + 4],
                                       1.0 / DM, rstd, ALU.mult, ALU.subtract)
        nc.scalar.activation(rstd, rstd, AF.Ln, bias=epsc[:])
        nc.scalar.activation(rstd, rstd, AF.Exp, scale=-0.5)
        for c in range(c0, c0 + 4):
            rc = res[:, c * DM:(c + 1) * DM]
            lnt = scratch.tile([128, DM], F16, tag="lnapply")
            nc.vector.tensor_scalar(lnt[:], rc, mu[:, c - c0:c - c0 + 1],
                                    rstd[:, c - c0:c - c0 + 1],
                                    ALU.subtract, ALU.mult)
            ptr = ps_tr.tile([128, DM], F16, tag="tr")
            for mc in range(NMC):
                nc.tensor.transpose(ptr[:, mc * 128:(mc + 1) * 128],
                                    lnt[:, mc * 128:(mc + 1) * 128],
                                    ident_sb[:])
            nc.scalar.copy(lnT3[:, :, c * 128:(c + 1) * 128],
                           ptr[:].rearrange("p (m t) -> p m t", m=NMC))

    # conv: accumulate taps into a per-half acc tile (DVE), then a
    # sigmoid chain on ACT (exp/ln only - no Silu table) and one TT for
    # xs = acc * sigmoid(acc), written back into xbuf (overlay)
    acch = {}

    tails = smalls.tile([128, NDC * 3], F16, tag="tails")

    def conv_acc(dc, hf):
        if dc == 0:
            ach = scr1.tile([128, NDC * H], F16, tag="et")
            acch[hf] = ach
        acc = acch[hf][:, dc * H:(dc + 1) * H]
        x0 = dc * TP3 + hf * H
        if hf == 0:
            # save raw x for tokens 509..511 before the xs overlay eats them
            nc.vector.tensor_copy(tails[:, dc * 3:(dc + 1) * 3],
                                  xbuf[:, x0 + H:x0 + H + 3])
        nc.vector.tensor_scalar(acc, xbuf[:, x0:x0 + H], convw_c(0, dc),
                                convb_c(dc), ALU.mult, ALU.add)
        for k in range(1, DCONV):
            nc.vector.scalar_tensor_tensor(acc, xbuf[:, x0 + k:x0 + k + H],
                                           convw_c(k, dc), acc,
                                           ALU.mult, ALU.add)
        if hf == 1:
            # first 3 outputs used xs-overlaid inputs; recompute them from
            # the saved tail (tokens 509-511) + raw cols (tokens 512-514)
            win = tiny.tile([128, 6], F16, tag="cwin")
            nc.vector.tensor_copy(win[:, 0:3], tails[:, dc * 3:(dc + 1) * 3])
            nc.vector.tensor_copy(win[:, 3:6], xbuf[:, x0 + 3:x0 + 6])
            a3 = acc[:, 0:3] if False else acch[hf][:, dc * H:dc * H + 3]
            nc.vector.tensor_scalar(a3, win[:, 0:3], convw_c(0, dc),
                                    convb_c(dc), ALU.mult, ALU.add)
            for k in range(1, DCONV):
                nc.vector.scalar_tensor_tensor(a3, win[:, k:k + 3],
                                               convw_c(k, dc), a3,
                                               ALU.mult, ALU.add)

    def conv_fin(hf):
        ach = acch[hf]
        nc.scalar.activation(
            _s3(xbuf, 3 + hf * H, TP3, NDC, H),
            ach[:].rearrange("p (k m) -> p k m", k=NDC), AF.Silu)

    def inproj_x(nh):
        for dc in range(NDC):
            pm = ps_mm.tile([128, 512], F32, tag="mm")
            for k in range(NMC):
                nc.tensor.matmul(
                    pm[:],
                    winx[:, k * DH + dc * 128: k * DH + (dc + 1) * 128],
                    lnT[:, k * T + nh * 512: k * T + (nh + 1) * 512],
                    start=(k == 0), stop=(k == NMC - 1))
            dst = xbuf[:, dc * TP3 + 3 + nh * 512:
                       dc * TP3 + 3 + (nh + 1) * 512]
            nc.scalar.activation(dst, pm[:], AF.Identity, bias=bx_c(dc))
            conv_acc(dc, nh)
        conv_fin(nh)

    def xs_sl(dc, hf):
        x0 = dc * TP3 + 3 + hf * H
        return xbuf[:, x0:x0 + H]

    # --- x_proj for one half -> pair allreduce -> dbc16 ---
    def xproj_half(hf):
        pm = ps_mm.tile([80, 512], F32, tag="mm")
        for k in range(NDC):
            nc.tensor.matmul(
                pm[:], wxp[:, k * 80:(k + 1) * 80], xs_sl(k, hf),
                start=(k == 0), stop=(k == NDC - 1))
        dbstg = scr1.tile([80, H], CC_DT, tag="dbstg")
        nc.scalar.copy(dbstg[:], pm[:])
        db_in = dram.tile([80, H], CC_DT, tag="db_in")
        db_out = dram.tile([80, H], CC_DT, tag="db_out")
        nc.sync.dma_start(db_in[:], dbstg[:])
        if SKIP_CC:
            nc.sync.dma_start(db_out[:], db_in[:])
        else:
            nc.gpsimd.collective_compute(
                "AllReduce", ALU.add,
                replica_groups=[[0, 1], [2, 3], [4, 5], [6, 7]],
                ins=[db_in.opt()], outs=[db_out.opt()])
        nc.sync.dma_start(dbc16[:, hf * H:(hf + 1) * H], db_out[:, :])
        return db_out

    def bcasts(hf, db_out):
        bcB = bcpool.tile([128, NST * H], F16, tag="bcB")
        bcC = bcpool.tile([128, NST * H], F16, tag="bcC")
        for n in range(NST):
            if BCAST_MODE == "dma":
                nc.sync.dma_start(
                    bcB[:, n * H:(n + 1) * H],
                    bass.AP(db_out.tensor,
                            db_out[RDT + n:RDT + n + 1, :].offset,
                            [[0, 128], [1, H]]))
                nc.sync.dma_start(
                    bcC[:, n * H:(n + 1) * H],
                    bass.AP(db_out.tensor,
                            db_out[RDT + NST + n:RDT + NST + n + 1, :].offset,
                            [[0, 128], [1, H]]))
            else:
                brow = tiny.tile([1, H], F16, tag="brow")
                nc.sync.dma_start(brow[:], db_out[RDT + n:RDT + n + 1, :])
                nc.gpsimd.partition_broadcast(bcB[:, n * H:(n + 1) * H],
                                              brow[:])
                crow = tiny.tile([1, H], F16, tag="brow")
                nc.sync.dma_start(crow[:],
                                  db_out[RDT + NST + n:RDT + NST + n + 1, :])
                nc.gpsimd.partition_broadcast(bcC[:, n * H:(n + 1) * H],
                                              crow[:])
        return bcB, bcC

    def dt_chain(hf):
        # dt_proj -> softplus -> dts (PE/ACT only). hf0 uses the (then
        # free) wide ps_y banks so the 6 matmuls don't ping-pong through
        # the single ps_mm buffer on the critical path; hf1 runs hidden
        # inside hf0's scan block and can take the slow path.
        et = scr1.tile([128, NDC * H], F16, tag="et")
        if hf == 0:
            pmw = ps_y.tile([128, NDC * H], F32, tag="ypsum")
            for dc in range(NDC):
                nc.tensor.matmul(pmw[:, dc * H:(dc + 1) * H],
                                 wdt[:, dc * 128:(dc + 1) * 128],
                                 dbc16[0:RDT, hf * H:(hf + 1) * H],
                                 start=True, stop=True)
            for dc in range(NDC):
                nc.scalar.activation(et[:, dc * H:(dc + 1) * H],
                                     pmw[:, dc * H:(dc + 1) * H], AF.Exp,
                                     bias=bdt_c(dc))
        else:
            for dc in range(NDC):
                pm = ps_mm.tile([128, 512], F32, tag="mm")
                nc.tensor.matmul(pm[:], wdt[:, dc * 128:(dc + 1) * 128],
                                 dbc16[0:RDT, hf * H:(hf + 1) * H],
                                 start=True, stop=True)
                nc.scalar.activation(et[:, dc * H:(dc + 1) * H], pm[:],
                                     AF.Exp, bias=bdt_c(dc))
        dts3h = _s3(dts, hf * H, T, NDC, H)
        nc.scalar.activation(dts3h, et[:].rearrange("p (k m) -> p k m", k=NDC),
                             AF.Ln, bias=1.0)

    # z in_proj matmul unit (evac with Identity; the sigmoid gate is
    # applied later with exp/ln ops - keeps one ACT table loaded)
    def z_unit(q):
        dc, nh = q // 2, q % 2
        pm = ps_mm.tile([128, 512], F32, tag="mm")
        for k in range(NMC):
            nc.tensor.matmul(
                pm[:],
                winz[:, k * DH + dc * 128: k * DH + (dc + 1) * 128],
                lnT[:, k * T + nh * 512: k * T + (nh + 1) * 512],
                start=(k == 0), stop=(k == NMC - 1))
        dst = zs[:, dc * T + nh * 512: dc * T + (nh + 1) * 512]
        nc.scalar.activation(dst, pm[:], AF.Silu, bias=bz_c(dc))

    ophf = {}

    def outproj_unit(hf, mc, yh):
        if mc == 0:
            opstg = stpool.tile([128, NMC * H], CC_DT, tag="opstg")
            ophf[hf] = opstg
        pm = ps_mm.tile([128, 512], F32, tag="mm")
        for k in range(NDC):
            nc.tensor.matmul(
                pm[:], wout[:, k * DM + mc * 128: k * DM + (mc + 1) * 128],
                yh[:, k * H:(k + 1) * H],
                start=(k == 0), stop=(k == NDC - 1))
        nc.scalar.copy(ophf[hf][:, mc * H:(mc + 1) * H], pm[:])

    def outproj_ar(hf):
        op_in = dram.tile([128, NMC * H], CC_DT, tag="op_in")
        op_out = dram.tile([128, NMC * H], CC_DT, tag="op_out")
        nc.sync.dma_start(op_in[:], ophf[hf][:])
        if SKIP_CC:
            nc.sync.dma_start(op_out[:], op_in[:])
        else:
            nc.gpsimd.collective_compute(
                "AllReduce", ALU.add,
                replica_groups=[[0, 1], [2, 3], [4, 5], [6, 7]],
                ins=[op_in.opt()], outs=[op_out.opt()])
        opc = yhpool.tile([128, NMC * H], CC_DT, tag="yh")
        nc.sync.dma_start(opc[:], op_out[:])
        ophf[hf] = opc

    def res_add_unit(hf, tcl):
        opc = ophf[hf]
        tcg = hf * 4 + tcl
        ptr = ps_tr.tile([128, DM], F16, tag="tr")
        for mc in range(NMC):
            nc.tensor.transpose(
                ptr[:, mc * 128:(mc + 1) * 128],
                opc[:, mc * H + tcl * 128: mc * H + (tcl + 1) * 128],
                ident_sb[:])
        trs = scratch.tile([128, DM], F16, tag="trs")
        nc.scalar.copy(trs[:], ptr[:])
        rsl = res[:, tcg * DM:(tcg + 1) * DM]
        nc.vector.tensor_tensor(rsl, rsl, trs[:], ALU.add)

    # emission order tuned for engine-queue overlap: both convs on DVE
    # (fills DVE idle while the x_proj ARs fly); Pool only does bcasts,
    # boundary fixes and the late-n uB/hC spills
    ln_half(0)
    inproj_x(0)
    if pending is not None:
        pending()
    db_out0 = xproj_half(0)
    dt_chain(0)
    bc0 = bcasts(0, db_out0)
    ln_half(1)
    inproj_x(1)
    db_out1 = xproj_half(1)
    for q in range(2 * NDC):
        z_unit(q)

    # load wout early (prefetch)
    wout = wpool.tile([128, NDC * DM], F16, tag="wbig")
    load3(wout[:], lt("wout_t"), NDC)

    yh_prev = None

    # PE ramp-filler schedule: independent work interleaved into the n-loop
    # so the Tensor engine stays continuously busy and ACT never blocks the
    # dA stream on an unmet dependency
    def filler(hf, n, yh):
        if hf == 0:
            if n == 10:
                dt_chain(1)
        else:
            if n in (0, 2, 4):
                outproj_unit(0, n, yh_prev)
                outproj_unit(0, n + 1, yh_prev)
            elif n == 5:
                outproj_ar(0)
            elif n in (8, 10):
                res_add_unit(0, n - 8)
                res_add_unit(0, n - 7)

    # --- per t-half scan block + out_proj + AR + residual add ---
    for hf in range(2):
        bcB, bcC = bc0 if hf == 0 else bcasts(1, db_out1)
        dts3h = _s3(dts, hf * H, T, NDC, H)
        uhalf = upool.tile([128, NDC * H], F16, tag="uh")
        u3h = uhalf[:].rearrange("p (k m) -> p k m", k=NDC)
        nc.vector.tensor_tensor(u3h, dts3h,
                                _s3(xbuf, 3 + hf * H, TP3, NDC, H), ALU.mult)
        yh = yhpool.tile([128, NDC * H], F16, tag="yh")
        yps = ps_y.tile([128, NDC * H], F32, tag="ypsum")
        # D * xs seed
        for dc in range(NDC):
            nc.tensor.matmul(yps[:, dc * H:(dc + 1) * H],
                             ddiag[:, dc * 128:(dc + 1) * 128],
                             xs_sl(dc, hf), start=True, stop=False)
        hcs = {}
        for n in range(NST):
            filler(hf, n, yh)
            da = sring.tile([128, NDC * H], F16, tag="sr")
            da3 = da[:].rearrange("p (k m) -> p k m", k=NDC)
            if asc is not None:
                nc.scalar.activation(da3, dts3h, AF.Exp, scale=float(asc[n]))
            else:
                for dc in range(NDC):
                    nc.scalar.activation(
                        da[:, dc * H:(dc + 1) * H],
                        dts[:, dc * T + hf * H: dc * T + (hf + 1) * H],
                        AF.Exp,
                        scale=amat[:, dc * NST + n:dc * NST + n + 1])
            ub = sring.tile([128, NDC * H], F16, tag="sr")
            ub3 = ub[:].rearrange("p (k m) -> p k m", k=NDC)
            eng_ub = nc.gpsimd if n >= NST - UB_POOL_N else nc.vector
            eng_ub.tensor_tensor(
                ub3, u3h, _rep_mid(bcB[:, n * H:(n + 1) * H], NDC), ALU.mult)
            hcol0 = n * NDC
            if hf == 0:
                # chunk starts reset to zero state via dA=0 boundaries
                nc.gpsimd.memset(_cols(da, H, H, NDC - 1), 0.0)
                init = 0.0
            else:
                # inject half-0 final states at the chunk boundaries
                fixp = tiny.tile([128, NDC - 1], F32, tag="fixp")
                nc.gpsimd.tensor_tensor(
                    fixp[:], _cols(da, H, H, NDC - 1),
                    hfin[:, hcol0 + 1:hcol0 + NDC], ALU.mult)
                nc.gpsimd.tensor_tensor(
                    _cols(ub, H, H, NDC - 1), _cols(ub, H, H, NDC - 1),
                    fixp[:], ALU.add)
                nc.gpsimd.memset(_cols(da, H, H, NDC - 1), 0.0)
                init = hfin[:, hcol0:hcol0 + 1]
            hall = sring.tile([128, NDC * H], F16, tag="sr")
            nc.vector.tensor_tensor_scan(hall[:], da[:], ub[:], init,
                                         ALU.mult, ALU.add)
            if hf == 0:
                nc.vector.tensor_copy(hfin[:, hcol0:hcol0 + NDC],
                                      _cols(hall, H - 1, H, NDC))
            hc = hcpool.tile([128, NDC * H], F16, tag="hc")
            hc3 = hc[:].rearrange("p (k m) -> p k m", k=NDC)
            eng_hc = nc.gpsimd if n >= NST - HC_POOL_N else nc.vector
            eng_hc.tensor_tensor(
                hc3, hall[:].rearrange("p (k m) -> p k m", k=NDC),
                _rep_mid(bcC[:, n * H:(n + 1) * H], NDC), ALU.mult)
            hcs[n] = hc
            if n % 2 == 1:
                # batched PE accumulation keeps the Tensor engine ramped
                for m in range(n - 1, n + 1):
                    for dc in range(NDC):
                        nc.tensor.matmul(yps[:, dc * H:(dc + 1) * H],
                                         ident_sb[:],
                                         hcs[m][:, dc * H:(dc + 1) * H],
                                         start=False,
                                         stop=(m == NST - 1))
                hcs = {}
        # y = ypsum * silu(z)
        ytmp = hcpool.tile([128, NDC * H], F16, tag="hc")
        nc.scalar.copy(ytmp[:], yps[:])
        nc.vector.tensor_tensor(
            yh[:].rearrange("p (k m) -> p k m", k=NDC),
            ytmp[:].rearrange("p (k m) -> p k m", k=NDC),
            _s3(zs, hf * H, T, NDC, H), ALU.mult)
        yh_prev = yh

    # hf1's out_proj + AR launch here; the residual adds are deferred into
    # the next layer's emission so DVE's in-order queue isn't blocked on
    # the AR flight at the layer boundary
    for mc in range(NMC):
        outproj_unit(1, mc, yh_prev)
    outproj_ar(1)

    def tail():
        for tcl in range(4):
            res_add_unit(1, tcl)
    return tail


_PROGRAM = None
_A_SCALARS = None


def kernel(**inputs):
    return kernel_ex(inputs)[0]


def kernel_ex(inputs, trace=False):
    global _PROGRAM, _A_SCALARS
    in_maps = prepare_host_inputs(inputs)
    if _PROGRAM is None:
        _A_SCALARS = detect_a_scalars(inputs["A_log"])
        _PROGRAM = build_program(_A_SCALARS)
    kwargs = {}
    if trace:
        kwargs = dict(trace=True)
    res = run_bass_kernel_spmd(_PROGRAM, in_maps,
                               core_ids=list(range(N_CORES)), **kwargs)
    out = np.empty((B, L, DM), np.float32)
    for b in range(B):
        out[b] = res.results[2 * b]["out"]
    return out, res


# revision 47
# speedup vs baseline: 1.1625x; 1.1625x over previous
"""Trainium2 Bass kernel for a 4-layer Mamba (BioSeqMixer) model.

Sharding: 8 cores = (batch 4) x (d_inner half 2). Each core runs the full
4-layer stack for one batch element over its 768-channel half of d_inner.
Cross-core traffic per layer: pair all-reduce of the x_proj partial
([80,512] f16, per t-half) and of the out_proj partial ([768,512] f16 per
t-half), pipelined against compute of the other t-half.

Scan block: channels on partitions, time on the free axis; 16 independent
tensor_tensor_scan recurrences (one per state n), merged full-width per
(state, t-half) with dA=0 chunk-boundary resets (half 0) or injected
half-0 final states (half 1). y = sum_n h*C is accumulated on the PE via
batched identity matmuls into PSUM (D*xs folded in via a diag(D) matmul).
conv runs on DVE (fills the x_proj-AR wait), h*C products go to the Pool
engine, LN stats run on the ACT accumulator per t-half, and each layer's
second-half residual add is deferred into the next layer's emission so
the out-AR flight never blocks DVE's in-order queue.
"""

import sys

sys.path.insert(0, "/opt/trn_rl_repo")

import numpy as np

import concourse.bass as bass
import concourse.bacc as bacc
import concourse.mybir as mybir
import concourse.tile as tile
from concourse.bass_utils import run_bass_kernel_spmd

# model dims
B, L = 4, 1024
DM, NL, VOCAB = 768, 4, 8
DI, NST, DCONV, RDT = 1536, 16, 4, 48
EPS = 1e-5

# per-core dims
T = L            # tokens per core (one batch element)
H = T // 2       # t-half length
DH = DI // 2     # d_inner half per core
NDC = DH // 128  # d-chunks (6)
NMC = DM // 128  # d_model chunks (6)
NTC = T // 128   # token chunks (8)
G = 3            # d-chunks per group
NG = NDC // G    # groups (2)
GW = G * H       # group width (1536)

N_CORES = 8

F32 = mybir.dt.float32
F16 = mybir.dt.float16
AF = mybir.ActivationFunctionType
ALU = mybir.AluOpType

# knobs
UB_POOL_N = 0    # uB on Pool gates the serial scan chain - keep on DVE
HC_POOL_N = 6    # h*C mults on Pool only feed the PE accs, safe to offload
CONV_DVE_HF0 = True  # conv for t-half 0 on DVE (fills pre-scan DVE idle)
BCAST_MODE = "pool"   # 'pool' | 'dma'
CC_DT = F16      # dtype for the out_proj / x_proj allreduce
SKIP_CC = False  # replace collectives with local copies (for TimelineSim)


def _np16(x):
    return np.ascontiguousarray(x, dtype=np.float16)


def _np32(x):
    return np.ascontiguousarray(x, dtype=np.float32)


def detect_a_scalars(A_log):
    """If A[d, n] is d-independent for every (layer, n), return the
    per-(layer, n) scalar values; else None."""
    A = -np.exp(np.asarray(A_log, np.float64))  # (NL, DI, NST)
    scal = np.zeros((NL, NST), np.float64)
    for l in range(NL):
        for j in range(NST):
            col = A[l][:, j]
            if np.max(np.abs(col - col[0])) > 1e-6 * max(1.0, abs(col[0])):
                return None
            scal[l, j] = col[0]
    return scal


def prepare_host_inputs(inputs):
    """Returns per-core input dicts (host-side weight prep + sharding)."""
    embed = np.asarray(inputs["embed"], np.float32)
    input_ids = np.asarray(inputs["input_ids"])
    in_proj_w = np.asarray(inputs["in_proj_w"], np.float32)
    conv_w = np.asarray(inputs["conv_w"], np.float32)
    conv_b = np.asarray(inputs["conv_b"], np.float32)
    x_proj_w = np.asarray(inputs["x_proj_w"], np.float32)
    dt_proj_w = np.asarray(inputs["dt_proj_w"], np.float32)
    dt_proj_b = np.asarray(inputs["dt_proj_b"], np.float32)
    A_log = np.asarray(inputs["A_log"], np.float32)
    Dp = np.asarray(inputs["D"], np.float32)
    out_proj_w = np.asarray(inputs["out_proj_w"], np.float32)
    norm_w = np.asarray(inputs["norm_w"], np.float32)
    norm_b = np.asarray(inputs["norm_b"], np.float32)
    norm_f_w = np.asarray(inputs["norm_f_w"], np.float32)
    norm_f_b = np.asarray(inputs["norm_f_b"], np.float32)

    hidden0 = embed[input_ids]  # (B, L, DM)

    per_half = [{}, {}]
    for h in (0, 1):
        S = slice(h * DH, (h + 1) * DH)
        winx_t = np.empty((NL, DM, DH), np.float16)
        winz_t = np.empty((NL, DM, DH), np.float16)
        bxz = np.empty((NL, 2, DH), np.float32)
        vecs_extra = np.empty((NL, 6, DH), np.float32)  # convb,bdt,convw0..3
        wxp_t = np.empty((NL, DH, 80), np.float16)
        wdt_t = np.empty((NL, RDT, DH), np.float16)
        amat = np.empty((NL, DH, NST), np.float32)
        ddiag = np.empty((NL, NDC * 128, 128), np.float16)
        wout_t = np.empty((NL, DH, DM), np.float16)
        for l in range(NL):
            wx_rows = in_proj_w[l][:DI][S]          # (DH, DM)
            wz_rows = in_proj_w[l][DI:][S]          # (DH, DM)
            winx_t[l] = _np16((wx_rows * norm_w[l][None, :]).T)
            winz_t[l] = _np16((wz_rows * norm_w[l][None, :]).T)
            bxz[l, 0] = wx_rows @ norm_b[l]
            bxz[l, 1] = wz_rows @ norm_b[l]
            vecs_extra[l, 0] = conv_b[l][S]
            vecs_extra[l, 1] = dt_proj_b[l][S]
            for k in range(DCONV):
                vecs_extra[l, 2 + k] = conv_w[l][S][:, k]
            wxp_t[l] = _np16(x_proj_w[l][:, S].T)   # (DH, 80)
            wdt_t[l] = _np16(dt_proj_w[l][S].T)     # (RDT, DH)
            amat[l] = -np.exp(A_log[l][S])          # (DH, NST)
            for dc in range(NDC):
                dvals = Dp[l][S][dc * 128:(dc + 1) * 128]
                ddiag[l, dc * 128:(dc + 1) * 128] = _np16(np.diag(dvals))
            wout_t[l] = _np16(out_proj_w[l][:, S].T)  # (DH, DM)
        per_half[h] = dict(
            winx_t=winx_t, winz_t=winz_t, bxz=bxz, vecs_extra=vecs_extra,
            wxp_t=wxp_t, wdt_t=wdt_t, amat=amat, ddiag=ddiag, wout_t=wout_t,
        )

    wfin = _np16(np.tile(norm_f_w[None, :], (128, 1)))
    bfin = _np16(np.tile(norm_f_b[None, :], (128, 1)))
    ident = _np16(np.eye(128))

    in_maps = []
    for r in range(N_CORES):
        b, h = r // 2, r % 2
        m = dict(per_half[h])
        m = {k: np.ascontiguousarray(v) for k, v in m.items()}
        m["hidden0"] = _np16(hidden0[b])
        m["wfin"] = wfin
        m["bfin"] = bfin
        m["ident"] = ident
        in_maps.append(m)
    return in_maps


def build_program(a_scalars=None):
    nc = bacc.Bacc("TRN2", target_bir_lowering=False, debug=False,
                   num_devices=N_CORES)

    dt_in = {}

    def din(name, shape, dt=F32):
        dt_in[name] = nc.dram_tensor(name, list(shape), dt,
                                     kind="ExternalInput").ap()
        return dt_in[name]

    din("hidden0", (T, DM), F16)
    din("winx_t", (NL, DM, DH), F16)
    din("winz_t", (NL, DM, DH), F16)
    din("bxz", (NL, 2, DH))
    din("vecs_extra", (NL, 6, DH))
    din("wxp_t", (NL, DH, 80), F16)
    din("wdt_t", (NL, RDT, DH), F16)
    din("amat", (NL, DH, NST))
    din("ddiag", (NL, NDC * 128, 128), F16)
    din("wout_t", (NL, DH, DM), F16)
    din("wfin", (128, DM), F16)
    din("bfin", (128, DM), F16)
    din("ident", (128, 128), F16)

    out_ap = nc.dram_tensor("out", [T, DM], F32, kind="ExternalOutput").ap()

    with tile.TileContext(nc) as tc:
        _body(nc, tc, dt_in, out_ap, a_scalars)

    nc.compile()
    return nc


def _body(nc, tc, din, out_ap, a_scalars):
    import contextlib
    with contextlib.ExitStack() as ctx:
        _body_inner(ctx, nc, tc, din, out_ap, a_scalars)


def _rep_mid(ap, n):
    """[P, w] AP -> [P, n, w] with a step-0 broadcast middle dim."""
    return bass.AP(ap.tensor, ap.offset, [ap.ap[0], [0, n], ap.ap[1]])


def _s3(t, base_off, stride, nch, w):
    """3D AP [128, nch, w] into a persist tile at base_off with chunk
    stride `stride`."""
    b = t[:, base_off:base_off + 1]
    return bass.AP(b.tensor, b.offset, [b.ap[0], [stride, nch], [1, w]])


def _cols(t, start, stride, count):
    """Strided single-column AP [128, count] (1 elem per step)."""
    b = t[:, start:start + 1]
    return bass.AP(b.tensor, b.offset, [b.ap[0], [stride, count]])


def _ln_stats(nc, res, stats, sq, epsc):
    """Per token-chunk mean (mu) and 1/std (rstd) via ACT accumulators."""
    for c in range(NTC):
        rc = res[:, c * DM:(c + 1) * DM]
        nc.scalar.activation(sq[:, :DM], rc, AF.Identity,
                             accum_out=stats[:, c:c + 1])
        nc.scalar.activation(sq[:, :DM], rc, AF.Square,
                             accum_out=stats[:, NTC + c:NTC + c + 1])
    mu = stats[:, 2 * NTC:3 * NTC]
    rstd = stats[:, 3 * NTC:4 * NTC]
    nc.vector.tensor_scalar(mu, stats[:, 0:NTC], 1.0 / DM, None, ALU.mult)
    nc.vector.tensor_tensor(rstd, mu, mu, ALU.mult)
    nc.vector.scalar_tensor_tensor(rstd, stats[:, NTC:2 * NTC], 1.0 / DM,
                                   rstd, ALU.mult, ALU.subtract)
    nc.scalar.activation(rstd, rstd, AF.Ln, bias=epsc[:])
    nc.scalar.activation(rstd, rstd, AF.Exp, scale=-0.5)
    return mu, rstd


def _body_inner(ctx, nc, tc, din, out_ap, a_scalars):
    E = ctx.enter_context

    # pools
    persist = E(tc.tile_pool(name="persist", bufs=1))
    wpool = E(tc.tile_pool(name="weights", bufs=2))
    wsmall = E(tc.tile_pool(name="wsmall", bufs=1))
    lnt_pool = E(tc.tile_pool(name="lnt", bufs=1))
    scratch = E(tc.tile_pool(name="scratch", bufs=2))
    scr1 = E(tc.tile_pool(name="scr1", bufs=1))
    bcpool = E(tc.tile_pool(name="bc", bufs=1))
    sring = E(tc.tile_pool(name="sring", bufs=4))   # da/ub/hall ring
    upool = E(tc.tile_pool(name="upool", bufs=1))
    hcpool = E(tc.tile_pool(name="hc", bufs=3))
    yhpool = E(tc.tile_pool(name="yh", bufs=1))
    stpool = E(tc.tile_pool(name="stg", bufs=1))
    smalls = E(tc.tile_pool(name="smalls", bufs=1))
    tiny = E(tc.tile_pool(name="tiny", bufs=4))
    ps_mm = E(tc.tile_pool(name="ps_mm", bufs=1, space="PSUM"))
    ps_tr = E(tc.tile_pool(name="ps_tr", bufs=1, space="PSUM"))
    ps_y = E(tc.tile_pool(name="ps_y", bufs=1, space="PSUM"))
    dram = E(tc.tile_pool(name="dram", bufs=2, space="DRAM"))

    # persistent tiles
    res = persist.tile([128, NTC * DM], F16, tag="res")     # residual [t,dm]
    xbuf = persist.tile([128, NDC * (T + 3)], F16, tag="xbuf")  # conv in/out
    zs = persist.tile([128, NDC * T], F16, tag="zs")        # silu(z)
    dts = persist.tile([128, NDC * T], F16, tag="dts")      # softplus dt
    dbc16 = persist.tile([80, T], F16, tag="dbc16")
    hfin = persist.tile([128, NST * NDC], F32, tag="hfin")  # h finals half0
    ident_sb = persist.tile([128, 128], F16, tag="ident")
    wfin_sb = persist.tile([128, DM], F16, tag="wfin")
    bfin_sb = persist.tile([128, DM], F16, tag="bfin")
    epsc = persist.tile([128, 1], F32, tag="epsc")

    nc.vector.memset(epsc[:], EPS)
    nc.sync.dma_start(ident_sb[:], din["ident"][:, :])
    nc.sync.dma_start(wfin_sb[:], din["wfin"][:, :])
    nc.sync.dma_start(bfin_sb[:], din["bfin"][:, :])

    # residual <- hidden0 ([T, DM] -> [128, (tc dm)])
    nc.sync.dma_start(
        res[:].rearrange("p (c m) -> p c m", c=NTC),
        din["hidden0"].rearrange("(c p) m -> p c m", p=128))

    # zero the 3-column conv pads once
    for dc in range(NDC):
        nc.vector.memset(xbuf[:, dc * (T + 3): dc * (T + 3) + 3], 0.0)

    kw = dict(res=res, xbuf=xbuf, zs=zs, dts=dts, dbc16=dbc16,
              hfin=hfin, ident_sb=ident_sb, epsc=epsc,
              wpool=wpool, wsmall=wsmall, lnt_pool=lnt_pool, scratch=scratch,
              scr1=scr1,
              bcpool=bcpool, sring=sring, upool=upool,
              hcpool=hcpool, yhpool=yhpool, stpool=stpool, smalls=smalls,
              tiny=tiny, ps_mm=ps_mm, ps_tr=ps_tr, ps_y=ps_y, dram=dram)
    pending = None
    for layer in range(NL):
        asc = None if a_scalars is None else a_scalars[layer]
        pending = _layer(nc, tc, din, layer, asc, pending, **kw)
    pending()

    # final layernorm -> out
    stats = smalls.tile([128, 4 * NTC], F32, tag="stats")
    sq = scratch.tile([128, DM], F16, tag="sq")
    mu, rstd = _ln_stats(nc, res, stats, sq, epsc)
    for c in range(NTC):
        rc = res[:, c * DM:(c + 1) * DM]
        ot = scr1.tile([128, DM], F32, tag="lnout")
        nc.vector.tensor_scalar(ot[:], rc, mu[:, c:c + 1], rstd[:, c:c + 1],
                                ALU.subtract, ALU.mult)
        nc.vector.tensor_tensor(ot[:], ot[:], wfin_sb[:], ALU.mult)
        nc.vector.tensor_tensor(ot[:], ot[:], bfin_sb[:], ALU.add)
        nc.sync.dma_start(out_ap[c * 128:(c + 1) * 128, :], ot[:])


def _layer(nc, tc, din, layer, asc, pending, *, res, xbuf, zs, dts, dbc16, hfin,
           ident_sb, epsc, wpool, wsmall, lnt_pool, scratch, scr1, bcpool,
           sring, upool, hcpool, yhpool, stpool, smalls, tiny,
           ps_mm, ps_tr, ps_y, dram):
    lt = lambda name: din[name][layer]
    TP3 = T + 3

    def load3(tile_ap, dram_ap, k):
        nc.sync.dma_start(
            tile_ap.rearrange("p (k m) -> p k m", k=k),
            dram_ap.rearrange("(k p) m -> p k m", p=128))

    # --- load weights to sbuf ---
    winx = wpool.tile([128, NMC * DH], F16, tag="wbig")
    load3(winx[:], lt("winx_t"), NMC)
    winz = wpool.tile([128, NMC * DH], F16, tag="wbig")
    load3(winz[:], lt("winz_t"), NMC)
    wxp = wsmall.tile([128, NDC * 80], F16, tag="wxp")
    load3(wxp[:], lt("wxp_t"), NDC)
    wdt = wsmall.tile([RDT, DH], F16, tag="wdt")
    nc.sync.dma_start(wdt[:], lt("wdt_t")[:, :])
    ddiag = wsmall.tile([128, NDC * 128], F16, tag="ddiag")
    load3(ddiag[:], lt("ddiag"), NDC)
    amat = None
    if asc is None:
        amat = wsmall.tile([128, NDC * NST], F32, tag="amat")
        load3(amat[:], lt("amat"), NDC)
    vecs = wsmall.tile([128, NDC * 8], F32, tag="vecs")
    # layout per dchunk: [bx, bz, convb, bdt, convw0..3]
    nc.sync.dma_start(
        vecs[:, 0:NDC * 2].rearrange("p (b k) -> p b k", b=2),
        lt("bxz").rearrange("b (k p) -> p b k", p=128))
    nc.sync.dma_start(
        vecs[:, NDC * 2:NDC * 8].rearrange("p (v k) -> p v k", v=6),
        lt("vecs_extra").rearrange("v (k p) -> p v k", p=128))
    bx_c = lambda dc: vecs[:, dc:dc + 1]
    bz_c = lambda dc: vecs[:, NDC + dc:NDC + dc + 1]
    convb_c = lambda dc: vecs[:, NDC * 2 + dc:NDC * 2 + dc + 1]
    bdt_c = lambda dc: vecs[:, NDC * 3 + dc:NDC * 3 + dc + 1]
    convw_c = lambda k, dc: vecs[:, NDC * (4 + k) + dc:NDC * (4 + k) + dc + 1]

    # --- layernorm stats + apply, per t-half (so next-layer work for the
    # first half can start while the second half's out-AR is in flight) ---
    stats = smalls.tile([128, 4 * NTC], F32, tag="stats")
    sq = scr1.tile([128, DM], F16, tag="sq")
    lnT = lnt_pool.tile([128, NMC * T], F16, tag="lnT")
    lnT3 = lnT[:].rearrange("p (m t) -> p m t", m=NMC)

    def ln_half(hf):
        c0 = hf * 4
        for c in range(c0, c0 + 4):
            rc = res[:, c * DM:(c + 1) * DM]
            nc.scalar.activation(sq[:, :DM], rc, AF.Identity,
                                 accum_out=stats[:, c:c + 1])
            nc.scalar.activation(sq[:, :DM], rc, AF.Square,
                                 accum_out=stats[:, NTC + c:NTC + c + 1])
        mu = stats[:, 2 * NTC + c0:2 * NTC + c0 + 4]
        rstd = stats[:, 3 * NTC + c0:3 * NTC + c0 + 4]
        nc.vector.tensor_scalar(mu, stats[:, c0:c0 + 4], 1.0 / DM, None,
                                ALU.mult)
        nc.vector.tensor_tensor(rstd, mu, mu, ALU.mult)
        nc.vector.scalar_tensor_tensor(rstd, stats[:, NTC + c0:NTC + c0 + 4],
                                       1.0 / DM, rstd, ALU.mult, ALU.subtract)
        nc.scalar.activation(rstd, rstd, AF.Ln, bias=epsc[:])
        nc.scalar.activation(rstd, rstd, AF.Exp, scale=-0.5)
        for c in range(c0, c0 + 4):
            rc = res[:, c * DM:(c + 1) * DM]
            lnt = scratch.tile([128, DM], F16, tag="lnapply")
            nc.vector.tensor_scalar(lnt[:], rc, mu[:, c - c0:c - c0 + 1],
                                    rstd[:, c - c0:c - c0 + 1],
                                    ALU.subtract, ALU.mult)
            ptr = ps_tr.tile([128, DM], F16, tag="tr")
            for mc in range(NMC):
                nc.tensor.transpose(ptr[:, mc * 128:(mc + 1) * 128],
                                    lnt[:, mc * 128:(mc + 1) * 128],
                                    ident_sb[:])
            nc.scalar.copy(lnT3[:, :, c * 128:(c + 1) * 128],
                           ptr[:].rearrange("p (m t) -> p m t", m=NMC))

    # conv: accumulate taps into a per-half acc tile (DVE), then a
    # sigmoid chain on ACT (exp/ln only - no Silu table) and one TT for
    # xs = acc * sigmoid(acc), written back into xbuf (overlay)
    acch = {}

    tails = smalls.tile([128, NDC * 3], F16, tag="tails")

    def conv_acc(dc, hf):
        if dc == 0:
            ach = scr1.tile([128, NDC * H], F16, tag="et")
            acch[hf] = ach
        acc = acch[hf][:, dc * H:(dc + 1) * H]
        x0 = dc * TP3 + hf * H
        if hf == 0:
            # save raw x for tokens 509..511 before the xs overlay eats them
            nc.vector.tensor_copy(tails[:, dc * 3:(dc + 1) * 3],
                                  xbuf[:, x0 + H:x0 + H + 3])
        nc.vector.tensor_scalar(acc, xbuf[:, x0:x0 + H], convw_c(0, dc),
                                convb_c(dc), ALU.mult, ALU.add)
        for k in range(1, DCONV):
            nc.vector.scalar_tensor_tensor(acc, xbuf[:, x0 + k:x0 + k + H],
                                           convw_c(k, dc), acc,
                                           ALU.mult, ALU.add)
        if hf == 1:
            # first 3 outputs used xs-overlaid inputs; recompute them from
            # the saved tail (tokens 509-511) + raw cols (tokens 512-514)
            win = tiny.tile([128, 6], F16, tag="cwin")
            nc.vector.tensor_copy(win[:, 0:3], tails[:, dc * 3:(dc + 1) * 3])
            nc.vector.tensor_copy(win[:, 3:6], xbuf[:, x0 + 3:x0 + 6])
            a3 = acc[:, 0:3] if False else acch[hf][:, dc * H:dc * H + 3]
            nc.vector.tensor_scalar(a3, win[:, 0:3], convw_c(0, dc),
                                    convb_c(dc), ALU.mult, ALU.add)
            for k in range(1, DCONV):
                nc.vector.scalar_tensor_tensor(a3, win[:, k:k + 3],
                                               convw_c(k, dc), a3,
                                               ALU.mult, ALU.add)

    def conv_fin(hf):
        ach = acch[hf]
        nc.scalar.activation(
            _s3(xbuf, 3 + hf * H, TP3, NDC, H),
            ach[:].rearrange("p (k m) -> p k m", k=NDC), AF.Silu)

    def inproj_x(nh):
        for dc in range(NDC):
            pm = ps_mm.tile([128, 512], F32, tag="mm")
            for k in range(NMC):
                nc.tensor.matmul(
                    pm[:],
                    winx[:, k * DH + dc * 128: k * DH + (dc + 1) * 128],
                    lnT[:, k * T + nh * 512: k * T + (nh + 1) * 512],
                    start=(k == 0), stop=(k == NMC - 1))
            dst = xbuf[:, dc * TP3 + 3 + nh * 512:
                       dc * TP3 + 3 + (nh + 1) * 512]
            nc.scalar.activation(dst, pm[:], AF.Identity, bias=bx_c(dc))
            conv_acc(dc, nh)
        conv_fin(nh)

    def xs_sl(dc, hf):
        x0 = dc * TP3 + 3 + hf * H
        return xbuf[:, x0:x0 + H]

    # --- x_proj for one half -> pair allreduce -> dbc16 ---
    def xproj_half(hf):
        pm = ps_mm.tile([80, 512], F32, tag="mm")
        for k in range(NDC):
            nc.tensor.matmul(
                pm[:], wxp[:, k * 80:(k + 1) * 80], xs_sl(k, hf),
                start=(k == 0), stop=(k == NDC - 1))
        dbstg = scr1.tile([80, H], CC_DT, tag="dbstg")
        nc.scalar.copy(dbstg[:], pm[:])
        db_in = dram.tile([80, H], CC_DT, tag="db_in")
        db_out = dram.tile([80, H], CC_DT, tag="db_out")
        nc.sync.dma_start(db_in[:], dbstg[:])
        if SKIP_CC:
            nc.sync.dma_start(db_out[:], db_in[:])
        else:
            nc.gpsimd.collective_compute(
                "AllReduce", ALU.add,
                replica_groups=[[0, 1], [2, 3], [4, 5], [6, 7]],
                ins=[db_in.opt()], outs=[db_out.opt()])
        nc.sync.dma_start(dbc16[:, hf * H:(hf + 1) * H], db_out[:, :])
        return db_out

    def bcasts(hf, db_out):
        bcB = bcpool.tile([128, NST * H], F16, tag="bcB")
        bcC = bcpool.tile([128, NST * H], F16, tag="bcC")
        for n in range(NST):
            if BCAST_MODE == "dma":
                nc.sync.dma_start(
                    bcB[:, n * H:(n + 1) * H],
                    bass.AP(db_out.tensor,
                            db_out[RDT + n:RDT + n + 1, :].offset,
                            [[0, 128], [1, H]]))
                nc.sync.dma_start(
                    bcC[:, n * H:(n + 1) * H],
                    bass.AP(db_out.tensor,
                            db_out[RDT + NST + n:RDT + NST + n + 1, :].offset,
                            [[0, 128], [1, H]]))
            else:
                brow = tiny.tile([1, H], F16, tag="brow")
                nc.sync.dma_start(brow[:], db_out[RDT + n:RDT + n + 1, :])
                nc.gpsimd.partition_broadcast(bcB[:, n * H:(n + 1) * H],
                                              brow[:])
                crow = tiny.tile([1, H], F16, tag="brow")
                nc.sync.dma_start(crow[:],
                                  db_out[RDT + NST + n:RDT + NST + n + 1, :])
                nc.gpsimd.partition_broadcast(bcC[:, n * H:(n + 1) * H],
                                              crow[:])
        return bcB, bcC

    def dt_chain(hf):
        # dt_proj -> softplus -> dts (PE/ACT only). hf0 uses the (then
        # free) wide ps_y banks so the 6 matmuls don't ping-pong through
        # the single ps_mm buffer on the critical path; hf1 runs hidden
        # inside hf0's scan block and can take the slow path.
        et = scr1.tile([128, NDC * H], F16, tag="et")
        if hf == 0:
            pmw = ps_y.tile([128, NDC * H], F32, tag="ypsum")
            for dc in range(NDC):
                nc.tensor.matmul(pmw[:, dc * H:(dc + 1) * H],
                                 wdt[:, dc * 128:(dc + 1) * 128],
                                 dbc16[0:RDT, hf * H:(hf + 1) * H],
                                 start=True, stop=True)
            for dc in range(NDC):
                nc.scalar.activation(et[:, dc * H:(dc + 1) * H],
                                     pmw[:, dc * H:(dc + 1) * H], AF.Exp,
                                     bias=bdt_c(dc))
        else:
            for dc in range(NDC):
                pm = ps_mm.tile([128, 512], F32, tag="mm")
                nc.tensor.matmul(pm[:], wdt[:, dc * 128:(dc + 1) * 128],
                                 dbc16[0:RDT, hf * H:(hf + 1) * H],
                                 start=True, stop=True)
                nc.scalar.activation(et[:, dc * H:(dc + 1) * H], pm[:],
                                     AF.Exp, bias=bdt_c(dc))
        dts3h = _s3(dts, hf * H, T, NDC, H)
        nc.scalar.activation(dts3h, et[:].rearrange("p (k m) -> p k m", k=NDC),
                             AF.Ln, bias=1.0)

    # z in_proj matmul unit (evac with Identity; the sigmoid gate is
    # applied later with exp/ln ops - keeps one ACT table loaded)
    def z_unit(q):
        dc, nh = q // 2, q % 2
        pm = ps_mm.tile([128, 512], F32, tag="mm")
        for k in range(NMC):
            nc.tensor.matmul(
                pm[:],
                winz[:, k * DH + dc * 128: k * DH + (dc + 1) * 128],
                lnT[:, k * T + nh * 512: k * T + (nh + 1) * 512],
                start=(k == 0), stop=(k == NMC - 1))
        dst = zs[:, dc * T + nh * 512: dc * T + (nh + 1) * 512]
        nc.scalar.activation(dst, pm[:], AF.Silu, bias=bz_c(dc))

    ophf = {}

    def outproj_unit(hf, mc, yh):
        if mc == 0:
            opstg = stpool.tile([128, NMC * H], CC_DT, tag="opstg")
            ophf[hf] = opstg
        pm = ps_mm.tile([128, 512], F32, tag="mm")
        for k in range(NDC):
            nc.tensor.matmul(
                pm[:], wout[:, k * DM + mc * 128: k * DM + (mc + 1) * 128],
                yh[:, k * H:(k + 1) * H],
                start=(k == 0), stop=(k == NDC - 1))
        nc.scalar.copy(ophf[hf][:, mc * H:(mc + 1) * H], pm[:])

    def outproj_ar(hf):
        op_in = dram.tile([128, NMC * H], CC_DT, tag="op_in")
        op_out = dram.tile([128, NMC * H], CC_DT, tag="op_out")
        nc.sync.dma_start(op_in[:], ophf[hf][:])
        if SKIP_CC:
            nc.sync.dma_start(op_out[:], op_in[:])
        else:
            nc.gpsimd.collective_compute(
                "AllReduce", ALU.add,
                replica_groups=[[0, 1], [2, 3], [4, 5], [6, 7]],
                ins=[op_in.opt()], outs=[op_out.opt()])
        opc = yhpool.tile([128, NMC * H], CC_DT, tag="yh")
        nc.sync.dma_start(opc[:], op_out[:])
        ophf[hf] = opc

    def res_add_unit(hf, tcl):
        opc = ophf[hf]
        tcg = hf * 4 + tcl
        ptr = ps_tr.tile([128, DM], F16, tag="tr")
        for mc in range(NMC):
            nc.tensor.transpose(
                ptr[:, mc * 128:(mc + 1) * 128],
                opc[:, mc * H + tcl * 128: mc * H + (tcl + 1) * 128],
                ident_sb[:])
        trs = scratch.tile([128, DM], F16, tag="trs")
        nc.scalar.copy(trs[:], ptr[:])
        rsl = res[:, tcg * DM:(tcg + 1) * DM]
        nc.vector.tensor_tensor(rsl, rsl, trs[:], ALU.add)

    # emission order tuned for engine-queue overlap: both convs on DVE
    # (fills DVE idle while the x_proj ARs fly); Pool only does bcasts,
    # boundary fixes and the late-n uB/hC spills
    ln_half(0)
    inproj_x(0)
    if pending is not None:
        pending()
    db_out0 = xproj_half(0)
    dt_chain(0)
    bc0 = bcasts(0, db_out0)
    ln_half(1)
    inproj_x(1)
    for q in range(2 * NDC):
        z_unit(q)
    db_out1 = xproj_half(1)

    # load wout early (prefetch)
    wout = wpool.tile([128, NDC * DM], F16, tag="wbig")
    load3(wout[:], lt("wout_t"), NDC)

    yh_prev = None

    # PE ramp-filler schedule: independent work interleaved into the n-loop
    # so the Tensor engine stays continuously busy and ACT never blocks the
    # dA stream on an unmet dependency
    def filler(hf, n, yh):
        if hf == 0:
            if n == 10:
                dt_chain(1)
        else:
            if n in (0, 2, 4):
                outproj_unit(0, n, yh_prev)
                outproj_unit(0, n + 1, yh_prev)
            elif n == 5:
                outproj_ar(0)
            elif n in (8, 10):
                res_add_unit(0, n - 8)
                res_add_unit(0, n - 7)

    # --- per t-half scan block + out_proj + AR + residual add ---
    for hf in range(2):
        bcB, bcC = bc0 if hf == 0 else bcasts(1, db_out1)
        dts3h = _s3(dts, hf * H, T, NDC, H)
        uhalf = upool.tile([128, NDC * H], F16, tag="uh")
        u3h = uhalf[:].rearrange("p (k m) -> p k m", k=NDC)
        nc.vector.tensor_tensor(u3h, dts3h,
                                _s3(xbuf, 3 + hf * H, TP3, NDC, H), ALU.mult)
        yh = yhpool.tile([128, NDC * H], F16, tag="yh")
        yps = ps_y.tile([128, NDC * H], F32, tag="ypsum")
        # D * xs seed
        for dc in range(NDC):
            nc.tensor.matmul(yps[:, dc * H:(dc + 1) * H],
                             ddiag[:, dc * 128:(dc + 1) * 128],
                             xs_sl(dc, hf), start=True, stop=False)
        hcs = {}
        for n in range(NST):
            filler(hf, n, yh)
            da = sring.tile([128, NDC * H], F16, tag="sr")
            da3 = da[:].rearrange("p (k m) -> p k m", k=NDC)
            if asc is not None:
                nc.scalar.activation(da3, dts3h, AF.Exp, scale=float(asc[n]))
            else:
                for dc in range(NDC):
                    nc.scalar.activation(
                        da[:, dc * H:(dc + 1) * H],
                        dts[:, dc * T + hf * H: dc * T + (hf + 1) * H],
                        AF.Exp,
                        scale=amat[:, dc * NST + n:dc * NST + n + 1])
            ub = sring.tile([128, NDC * H], F16, tag="sr")
            ub3 = ub[:].rearrange("p (k m) -> p k m", k=NDC)
            eng_ub = nc.gpsimd if n >= NST - UB_POOL_N else nc.vector
            eng_ub.tensor_tensor(
                ub3, u3h, _rep_mid(bcB[:, n * H:(n + 1) * H], NDC), ALU.mult)
            hcol0 = n * NDC
            if hf == 0:
                # chunk starts reset to zero state via dA=0 boundaries
                nc.gpsimd.memset(_cols(da, H, H, NDC - 1), 0.0)
                init = 0.0
            else:
                # inject half-0 final states at the chunk boundaries
                fixp = tiny.tile([128, NDC - 1], F32, tag="fixp")
                nc.gpsimd.tensor_tensor(
                    fixp[:], _cols(da, H, H, NDC - 1),
                    hfin[:, hcol0 + 1:hcol0 + NDC], ALU.mult)
                nc.gpsimd.tensor_tensor(
                    _cols(ub, H, H, NDC - 1), _cols(ub, H, H, NDC - 1),
                    fixp[:], ALU.add)
                nc.gpsimd.memset(_cols(da, H, H, NDC - 1), 0.0)
                init = hfin[:, hcol0:hcol0 + 1]
            hall = sring.tile([128, NDC * H], F16, tag="sr")
            nc.vector.tensor_tensor_scan(hall[:], da[:], ub[:], init,
                                         ALU.mult, ALU.add)
            if hf == 0:
                nc.vector.tensor_copy(hfin[:, hcol0:hcol0 + NDC],
                                      _cols(hall, H - 1, H, NDC))
            hc = hcpool.tile([128, NDC * H], F16, tag="hc")
            hc3 = hc[:].rearrange("p (k m) -> p k m", k=NDC)
            eng_hc = nc.gpsimd if n in (4, 6, 8, 10, 12, 14) else nc.vector
            eng_hc.tensor_tensor(
                hc3, hall[:].rearrange("p (k m) -> p k m", k=NDC),
                _rep_mid(bcC[:, n * H:(n + 1) * H], NDC), ALU.mult)
            hcs[n] = hc
            if n % 2 == 1:
                # batched PE accumulation keeps the Tensor engine ramped
                for m in range(n - 1, n + 1):
                    for dc in range(NDC):
                        nc.tensor.matmul(yps[:, dc * H:(dc + 1) * H],
                                         ident_sb[:],
                                         hcs[m][:, dc * H:(dc + 1) * H],
                                         start=False,
                                         stop=(m == NST - 1))
                hcs = {}
        # y = ypsum * silu(z)
        ytmp = hcpool.tile([128, NDC * H], F16, tag="hc")
        nc.scalar.copy(ytmp[:], yps[:])
        nc.vector.tensor_tensor(
            yh[:].rearrange("p (k m) -> p k m", k=NDC),
            ytmp[:].rearrange("p (k m) -> p k m", k=NDC),
            _s3(zs, hf * H, T, NDC, H), ALU.mult)
        yh_prev = yh

    # hf1's out_proj + AR launch here; the residual adds are deferred into
    # the next layer's emission so DVE's in-order queue isn't blocked on
    # the AR flight at the layer boundary
    for mc in range(NMC):
        outproj_unit(1, mc, yh_prev)
    outproj_ar(1)

    def tail():
        for tcl in range(4):
            res_add_unit(1, tcl)
    return tail


_PROGRAM = None
_A_SCALARS = None


def kernel(**inputs):
    return kernel_ex(inputs)[0]


def kernel_ex(inputs, trace=False):
    global _PROGRAM, _A_SCALARS
    in_maps = prepare_host_inputs(inputs)
    if _PROGRAM is None:
        _A_SCALARS = detect_a_scalars(inputs["A_log"])
        _PROGRAM = build_program(_A_SCALARS)
    kwargs = {}
    if trace:
        kwargs = dict(trace=True)
    res = run_bass_kernel_spmd(_PROGRAM, in_maps,
                               core_ids=list(range(N_CORES)), **kwargs)
    out = np.empty((B, L, DM), np.float32)
    for b in range(B):
        out[b] = res.results[2 * b]["out"]
    return out, res
